# revision 1
# baseline (speedup 1.0000x reference)
"""Trainium2 Bass kernel for nn_MultiHeadDuelingDQN (8-core SPMD).

Model (B=256, STATE=26240, H=512, R=4000, N=64 heads, M=10):
    h  = relu(relu(x@W1+b1)@W2+b2)
    q_cache = h@Wvc+bvc + (h@Wac+bac) - mean_R(h@Wac+bac)
    q_assoc = per-head dueling over M (local means)
    q_rec   = S - mean_R(S),  S = sum_n (h@Wru[n]+bru[n])   [exact rewrite:
              rec_global has zero row-mean, so the reference's second mean
              subtraction is a no-op and S never needs the [B,N,R] tensor]

Sharding (8 cores):
  - fc1: contraction (STATE) split 8 ways; partial h1_pre [256,512] exchanged
    via AllToAll (cheapest collective here) + local 8-way sum of the core's
    32 batch rows (b1/8 folded pre-exchange, relu post-sum); fc2 computed on
    the 32 local rows, then AllGather replicates h2 to all cores.
  - rec/cache: R split 8 ways (500 cols/core); the sum over heads becomes a
    DVE reduction over repacked contiguous Wru supertiles ([128, 4*500] per
    DMA, heads interleaved innermost), then one small matmul h @ W_sum.
    Row-means over the full R use a tiny [128,4] AllGather + local reduce.
  - assoc heads: split 8 ways (8 heads/core), fully local; augmented matmul
    [Wau | Wvu | Wvc] -> [adv_assoc | val_n | value_c] in one pass.
Engine split: PE does transposes + all matmuls; DVE is dedicated to the Wru
stream reduction; ACT (scalar) does PSUM->SBUF copies, relus, row-sums
(accum_out) and mean subtraction (Identity+bias), plus non-stream DMA
dispatch; Sync dispatches the Wru stream; GpSimd runs collectives and small
SBUF elementwise ops.

kernel(**inputs) takes full unsharded inputs, returns full [256, 8640].
"""
import os
os.environ.setdefault("NEURON_RT_DBG_RDH_CC", "0")

import numpy as np

import concourse.bass as bass
import concourse.mybir as mybir
import concourse.tile as tile
from concourse import bacc
from concourse import bass_utils
from concourse.bass import ts
from concourse.masks import make_identity

NC = 8
B, H, STATE, R, NH, M = 256, 512, 26240, 4000, 64, 10
KPC_RAW = STATE // NC          # 3280
KCH = 26                       # k-chunks of 128 per core (padded)
KPC = KCH * 128                # 3328
RPC = R // NC                  # 500
HPC = NH // NC                 # 8 heads per core
AUG = HPC * (M + 1) + 1        # 89 = [8x(10 adv + 1 val)] + value_c
GRP = 4                        # heads per wru supertile
NGRP = NH // GRP               # 8 supertiles per k-chunk
W1GROUPS = [(0, 4), (4, 4), (8, 4), (12, 4), (16, 4), (20, 4), (24, 2)]
F32 = mybir.dt.float32
RELU = mybir.ActivationFunctionType.Relu
COPY = mybir.ActivationFunctionType.Copy
IDENT = mybir.ActivationFunctionType.Identity
ADD = mybir.AluOpType.add
SUB = mybir.AluOpType.subtract


def build_program(wru_bufs=7):
    nc = bacc.Bacc("TRN2", target_bir_lowering=False, debug=False, num_devices=NC)

    # ---- per-core I/O ----
    xs = nc.dram_tensor("xs", [B, KPC], F32, kind="ExternalInput").ap()
    w1g = [nc.dram_tensor(f"w1g{i}", [128, 512 * L], F32, kind="ExternalInput").ap()
           for i, (_, L) in enumerate(W1GROUPS)]
    b1 = nc.dram_tensor("b1", [H], F32, kind="ExternalInput").ap()
    w2 = nc.dram_tensor("w2", [H, H], F32, kind="ExternalInput").ap()
    b2 = nc.dram_tensor("b2", [H], F32, kind="ExternalInput").ap()
    wac = nc.dram_tensor("wac", [H, RPC], F32, kind="ExternalInput").ap()
    bac = nc.dram_tensor("bac", [RPC], F32, kind="ExternalInput").ap()
    # repacked r-major: [kc, grp, p, r*8+gi] = Wru[grp*8+gi, kc*128+p, r0+r]
    wru = nc.dram_tensor("wru", [4, NGRP, 128, GRP * RPC], F32,
                         kind="ExternalInput").ap()
    bru = nc.dram_tensor("bru", [NH, RPC], F32, kind="ExternalInput").ap()
    wau = nc.dram_tensor("wau", [HPC, H, M], F32, kind="ExternalInput").ap()
    bau = nc.dram_tensor("bau", [HPC, M], F32, kind="ExternalInput").ap()
    wvu = nc.dram_tensor("wvu", [HPC, H], F32, kind="ExternalInput").ap()
    bvu = nc.dram_tensor("bvu", [HPC], F32, kind="ExternalInput").ap()
    wvc = nc.dram_tensor("wvc", [H], F32, kind="ExternalInput").ap()
    bvc = nc.dram_tensor("bvc", [1], F32, kind="ExternalInput").ap()

    out_cache = nc.dram_tensor("out_cache", [B, RPC], F32, kind="ExternalOutput").ap()
    out_rec = nc.dram_tensor("out_rec", [B, RPC], F32, kind="ExternalOutput").ap()
    out_assoc = nc.dram_tensor("out_assoc", [B, HPC * M], F32, kind="ExternalOutput").ap()

    with tile.TileContext(nc) as tc:
        with (
            tc.tile_pool(name="cst", bufs=1) as cst,
            tc.tile_pool(name="sb", bufs=1) as sb,
            tc.tile_pool(name="w1p", bufs=3) as w1p,
            tc.tile_pool(name="wrup", bufs=wru_bufs) as wrup,
            tc.tile_pool(name="partp", bufs=1) as partp,
            tc.tile_pool(name="ps", bufs=2, space="PSUM") as ps,
            tc.tile_pool(name="psfc", bufs=2, space="PSUM") as psfc,
            tc.tile_pool(name="dram", bufs=1, space="DRAM") as dram,
        ):
            ident = cst.tile([128, 128], F32, tag="ident")
            make_identity(nc, ident)
            ones8 = cst.tile([1, 128], F32, tag="ones8")
            nc.vector.memset(ones8, 1.0 / NC)
            ones1 = cst.tile([1, 128], F32, tag="ones1")
            nc.vector.memset(ones1, 1.0)
            ones64 = cst.tile([64, 128], F32, tag="ones64")
            nc.vector.memset(ones64, 1.0)

            # x slice loads first (they gate the whole trunk)
            x_sb = []
            for bt in range(2):
                xsb = sb.tile([128, KPC], F32, tag=f"x_sb{bt}", name=f"x_sb{bt}")
                nc.scalar.dma_start(xsb, xs[ts(bt, 128), :])
                x_sb.append(xsb)

            # ---------- Phase D emit first: Wru stream + head pre-sum (DVE) ----
            # (emission order does not pin execution order, but DMAs here only
            # depend on pool slots so they can start immediately)
            acc = [sb.tile([128, RPC], F32, tag=f"acc{k}", name=f"acc{k}")
                   for k in range(4)]
            for kc in range(4):
                for g in range(NGRP):
                    wt = wrup.tile([128, GRP * RPC], F32, tag="wru", name=f"wru_t{kc}_{g}")
                    nc.sync.dma_start(wt, wru[kc, g])
                    view = bass.AP(wt.tensor, wt.offset,
                                   [wt.ap[0], [GRP, RPC], [1, GRP]])
                    if g == 0:
                        nc.vector.tensor_reduce(acc[kc], view,
                                                axis=mybir.AxisListType.X, op=ADD)
                    else:
                        part = partp.tile([128, RPC], F32, tag="part",
                                          name=f"part{kc}_{g}")
                        nc.vector.tensor_reduce(part, view,
                                                axis=mybir.AxisListType.X, op=ADD)
                        nc.vector.tensor_add(acc[kc], acc[kc], part)
            bru_sb = sb.tile([64, RPC], F32, tag="bru_sb")
            nc.scalar.dma_start(bru_sb, bru)

            # ---------- Phase A: trunk ----------
            # fc1 pipeline: per k-chunk transpose x (both halves) then matmul
            b1row = cst.tile([1, H], F32, tag="b1row")
            nc.scalar.dma_start(b1row, b1.rearrange("(a h) -> a h", a=1))
            h1_ps = [psfc.tile([128, H], F32, tag="fc", name=f"h1_ps{bt}")
                     for bt in range(2)]
            for bt in range(2):  # fold b1/8 first, opens the accumulation group
                nc.tensor.matmul(h1_ps[bt], ones8, b1row, start=True, stop=False)
            for gi, (base, L) in enumerate(W1GROUPS):
                w1t = w1p.tile([128, 512 * L], F32, tag="w1g", name=f"w1t{gi}")
                nc.scalar.dma_start(w1t, w1g[gi])
                for j in range(L):
                    kc = base + j
                    xTt = sb.tile([128, B], F32, tag="xTrot", bufs=10,
                                  name=f"xT{kc}")
                    for bt in range(2):
                        pt = ps.tile([128, 128], F32, tag="small", bufs=4,
                                     name=f"ptx{bt}_{kc}")
                        nc.tensor.transpose(pt, x_sb[bt][:, ts(kc, 128)], ident)
                        nc.scalar.copy(xTt[:, ts(bt, 128)], pt)
                    for bt in range(2):
                        nc.tensor.matmul(h1_ps[bt], xTt[:, ts(bt, 128)],
                                         w1t[:, ts(j, 512)],
                                         start=False, stop=(kc == KCH - 1))

            # AllToAll h1_pre [256,512]: rank c receives 8 partials of its
            # 32 batch rows, then sums them locally (cheaper than RS here)
            BPC = B // NC  # 32 batch rows per core
            rs_in = dram.tile([B, H], F32, tag="rs_in")
            rs_out = dram.tile([B, H], F32, tag="rs_out")
            for bt in range(2):
                t = sb.tile([128, H], F32, tag=f"h1c{bt}", name=f"h1c{bt}")
                nc.scalar.copy(t, h1_ps[bt])
                nc.scalar.dma_start(rs_in[ts(bt, 128), :], t)
            nc.gpsimd.collective_compute(
                "AllToAll", mybir.AluOpType.bypass,
                replica_groups=[list(range(NC))],
                ins=[rs_in.opt()], outs=[rs_out.opt()],
            )
            # readback the 8 partials and tree-sum on GpSimd
            parts = []
            for i in range(NC):
                pti = sb.tile([BPC, H], F32, tag=f"h1p{i}", name=f"h1p{i}")
                nc.scalar.dma_start(pti, rs_out[ts(i, BPC), :])
                parts.append(pti)
            h1rs = sb.tile([BPC, H], F32, tag="h1rs")
            nc.gpsimd.tensor_add(h1rs, parts[0], parts[1])
            for i in range(2, NC):
                nc.gpsimd.tensor_add(h1rs, h1rs, parts[i])
            h1s = sb.tile([BPC, H], F32, tag="h1s")
            nc.scalar.activation(h1s, h1rs, RELU)
            h1cT = []
            for kc in range(4):
                pt = ps.tile([128, BPC], F32, tag="small", bufs=4, name=f"pth{kc}")
                nc.tensor.transpose(pt, h1s[:, ts(kc, 128)], ident[0:BPC, 0:BPC])
                t = sb.tile([128, BPC], F32, tag=f"h1cT{kc}", name=f"h1cT{kc}")
                nc.scalar.copy(t, pt)
                h1cT.append(t)
            w2t = []
            for kc in range(4):
                t = sb.tile([128, H], F32, tag=f"w2_{kc}", name=f"w2_{kc}")
                nc.scalar.dma_start(t, w2[ts(kc, 128), :])
                w2t.append(t)
            b2row = cst.tile([1, H], F32, tag="b2row")
            nc.scalar.dma_start(b2row, b2.rearrange("(a h) -> a h", a=1))
            h2_ps = psfc.tile([BPC, H], F32, tag="fc", name="h2_ps")
            nc.tensor.matmul(h2_ps, ones1[:, 0:BPC], b2row, start=True, stop=False)
            for kc in range(4):
                nc.tensor.matmul(h2_ps, h1cT[kc], w2t[kc],
                                 start=False, stop=(kc == 3))
            h2s = sb.tile([BPC, H], F32, tag="h2s")
            nc.scalar.activation(h2s, h2_ps, RELU)
            ag_in = dram.tile([BPC, H], F32, tag="ag_in")
            ag_out = dram.tile([B, H], F32, tag="ag_out")
            nc.scalar.dma_start(ag_in, h2s)
            nc.gpsimd.collective_compute(
                "AllGather", mybir.AluOpType.bypass,
                replica_groups=[list(range(NC))],
                ins=[ag_in.opt()], outs=[ag_out.opt()],
            )
            # h2 [256, 512] -> hT chunks [128(h2), 256(b)]
            hT = [sb.tile([128, B], F32, tag=f"hT{kc}", name=f"hT{kc}")
                  for kc in range(4)]
            for bt in range(2):
                h2g = sb.tile([128, H], F32, tag=f"h2g{bt}", name=f"h2g{bt}")
                nc.scalar.dma_start(h2g, ag_out[ts(bt, 128), :])
                for kc in range(4):
                    pt = ps.tile([128, 128], F32, tag="small", bufs=4, name=f"ptg{bt}_{kc}")
                    nc.tensor.transpose(pt, h2g[:, ts(kc, 128)], ident)
                    nc.scalar.copy(hT[kc][:, ts(bt, 128)], pt)

            # ---------- Phase B: assoc heads (augmented [adv|val|value_c]) ------
            aug_w = []
            for kc in range(4):
                t = cst.tile([128, AUG], F32, tag=f"aug_w{kc}", name=f"aug_w{kc}")
                grid = t[:, 0:HPC * (M + 1)].rearrange("p (n u) -> p n u", u=M + 1)
                nc.scalar.dma_start(
                    grid[:, :, 0:M],
                    wau[:, ts(kc, 128), :].rearrange("n k m -> k n m"))
                nc.scalar.dma_start(
                    grid[:, :, M:M + 1],
                    wvu[:, ts(kc, 128)].rearrange("n (k u) -> k n u", u=1))
                nc.scalar.dma_start(
                    t[:, AUG - 1:AUG],
                    wvc[ts(kc, 128)].rearrange("(k u) -> k u", u=1))
                aug_w.append(t)
            aug_b = cst.tile([1, AUG], F32, tag="aug_b")
            bgrid = aug_b[:, 0:HPC * (M + 1)].rearrange("p (n u) -> p n u", u=M + 1)
            nc.scalar.dma_start(bgrid[:, :, 0:M], bau.rearrange("n (a m) -> a n m", a=1))
            nc.scalar.dma_start(bgrid[:, :, M:M + 1],
                              bvu.rearrange("(a n u) -> a n u", a=1, u=1))
            nc.scalar.dma_start(aug_b[:, AUG - 1:AUG], bvc.rearrange("(a u) -> a u", a=1))

            value_sb = []
            junkA = sb.tile([128, M], F32, tag="junkA")
            for bt in range(2):
                psA = ps.tile([128, AUG], F32, tag="wide", name=f"psA{bt}")
                nc.tensor.matmul(psA, ones1, aug_b, start=True, stop=False)
                for kc in range(4):
                    nc.tensor.matmul(psA, hT[kc][:, ts(bt, 128)], aug_w[kc],
                                     start=False, stop=(kc == 3))
                # copy to SBUF so GpSimd can finalize (no DVE involvement)
                psA_sb = sb.tile([128, AUG], F32, tag=f"psAsb{bt}", name=f"psAsb{bt}")
                nc.scalar.copy(psA_sb, psA)
                advs = psA_sb[:, 0:HPC * (M + 1)].rearrange("p (n u) -> p n u", u=M + 1)
                # per-head -mean over M via ACT accum_out (free-axis sum)
                negm = sb.tile([128, HPC], F32, tag=f"negmA{bt}", name=f"negmA{bt}")
                for n in range(HPC):
                    nc.scalar.activation(junkA, advs[:, n, 0:M], COPY,
                                         scale=-1.0 / M,
                                         accum_out=negm[:, n:n + 1])
                tmp = sb.tile([128, HPC], F32, tag=f"tmpA{bt}", name=f"tmpA{bt}")
                nc.gpsimd.tensor_add(tmp, advs[:, :, M], negm)
                q = sb.tile([128, HPC * M], F32, tag=f"qA{bt}", name=f"qA{bt}")
                nc.gpsimd.tensor_tensor(
                    out=q.rearrange("p (n m) -> p n m", m=M),
                    in0=advs[:, :, 0:M],
                    in1=tmp.broadcast_to([128, HPC, M]),
                    op=ADD)
                nc.scalar.dma_start(out_assoc[ts(bt, 128), :], q)
                value_sb.append(psA_sb[:, AUG - 1:AUG])

            # ---------- Phase C: cache head (R-slice) ----------
            ar2_in = sb.tile([128, 4], F32, tag="ar2_in")
            wac_t = []
            for kc in range(4):
                t = sb.tile([128, RPC], F32, tag=f"wac{kc}", name=f"wac{kc}")
                nc.scalar.dma_start(t, wac[ts(kc, 128), :])
                wac_t.append(t)
            bac_sb = cst.tile([1, RPC], F32, tag="bac_sb")
            nc.scalar.dma_start(bac_sb, bac.rearrange("(a r) -> a r", a=1))
            adv_c_sb = []
            for bt in range(2):
                psC = ps.tile([128, RPC], F32, tag="wide", name=f"psC{bt}")
                nc.tensor.matmul(psC, ones1, bac_sb, start=True, stop=False)
                for kc in range(4):
                    nc.tensor.matmul(psC, hT[kc][:, ts(bt, 128)], wac_t[kc],
                                     start=False, stop=(kc == 3))
                t = sb.tile([128, RPC], F32, tag=f"advc{bt}", name=f"advc{bt}")
                # copy + row-sum in one ACT pass (accum_out)
                nc.scalar.activation(t, psC, COPY,
                                     accum_out=ar2_in[:, bt:bt + 1])
                adv_c_sb.append(t)

            # ---------- S = hT.T @ W_sum (+ sum_n bru fold), row-sums ----------
            psS, s_sb = [], []
            for bt in range(2):
                t = ps.tile([128, RPC], F32, tag="wide", name=f"psS{bt}")
                nc.tensor.matmul(t, ones64, bru_sb, start=True, stop=False)
                for kc in range(4):
                    nc.tensor.matmul(t, hT[kc][:, ts(bt, 128)], acc[kc],
                                     start=False, stop=(kc == 3))
                st = sb.tile([128, RPC], F32, tag=f"ssb{bt}", name=f"ssb{bt}")
                nc.scalar.activation(st, t, COPY,
                                     accum_out=ar2_in[:, 2 + bt:3 + bt])
                s_sb.append(st)
                psS.append(t)

            # ---------- Phase E: tiny AllReduce of row-sums, finalize ----------
            ar2_din = dram.tile([128, 4], F32, tag="ar2_din")
            ar2_dout = dram.tile([NC * 128, 4], F32, tag="ar2_dout")
            nc.scalar.dma_start(ar2_din, ar2_in)
            nc.gpsimd.collective_compute(
                "AllGather", mybir.AluOpType.bypass,
                replica_groups=[list(range(NC))],
                ins=[ar2_din.opt()], outs=[ar2_dout.opt()],
            )
            # one strided readback [128, (g,c)] then a single X-reduce over g
            rall = sb.tile([128, NC * 4], F32, tag="rall")
            nc.scalar.dma_start(rall, ar2_dout.rearrange("(g p) c -> p g c", p=128))
            rview = bass.AP(rall.tensor, rall.offset,
                            [rall.ap[0], [1, 4], [4, NC]])
            ar2_sb = sb.tile([128, 4], F32, tag="ar2_sb")
            nc.vector.tensor_reduce(ar2_sb, rview, axis=mybir.AxisListType.X, op=ADD)
            negmeans = sb.tile([128, 4], F32, tag="negmeans")
            nc.scalar.activation(negmeans, ar2_sb, COPY, scale=-1.0 / R)

            for bt in range(2):
                vm = sb.tile([128, 1], F32, tag=f"vm{bt}", name=f"vm{bt}")
                nc.gpsimd.tensor_add(vm, value_sb[bt], negmeans[:, bt:bt + 1])
                qc = sb.tile([128, RPC], F32, tag=f"qc{bt}", name=f"qc{bt}")
                nc.scalar.activation(qc, adv_c_sb[bt], IDENT, bias=vm, scale=1.0)
                nc.scalar.dma_start(out_cache[ts(bt, 128), :], qc)

                qr = sb.tile([128, RPC], F32, tag=f"qr{bt}", name=f"qr{bt}")
                nc.scalar.activation(qr, s_sb[bt], IDENT,
                                     bias=negmeans[:, 2 + bt:3 + bt], scale=1.0)
                nc.scalar.dma_start(out_rec[ts(bt, 128), :], qr)

    nc.compile()
    return nc


_CACHED = None


def _get_program():
    global _CACHED
    if _CACHED is None:
        _CACHED = build_program()
    return _CACHED


def make_in_maps(x, W1, b1, W2, b2, Wvc, bvc, Wac, bac, Wvu, bvu, Wau, bau, Wru, bru):
    f = np.float32
    x = np.asarray(x, f)
    W1 = np.asarray(W1, f)
    Wru = np.asarray(Wru, f)
    in_maps = []
    for c in range(NC):
        k0 = c * KPC_RAW
        xs = np.zeros((B, KPC), f)
        xs[:, :KPC_RAW] = x[:, k0:k0 + KPC_RAW]
        w1s = np.zeros((KPC, H), f)
        w1s[:KPC_RAW] = W1[k0:k0 + KPC_RAW]
        w1r = w1s.reshape(KCH, 128, H)
        r0 = c * RPC
        h0 = c * HPC
        # wru repack r-major: [kc, grp, p, r*8+gi] = Wru[grp*8+gi, kc*128+p, r0+r]
        ws = Wru[:, :, r0:r0 + RPC]                       # [64, 512, 500]
        a = ws.reshape(NGRP, GRP, 4, 128, RPC)            # [grp, gi, kc, p, r]
        wru_p = np.ascontiguousarray(a.transpose(2, 0, 3, 4, 1)).reshape(
            4, NGRP, 128, GRP * RPC)
        m = {
            "xs": xs,
            "b1": np.asarray(b1, f), "w2": np.asarray(W2, f), "b2": np.asarray(b2, f),
            "wac": np.ascontiguousarray(np.asarray(Wac, f)[:, r0:r0 + RPC]),
            "bac": np.ascontiguousarray(np.asarray(bac, f)[r0:r0 + RPC]),
            "wru": wru_p,
            "bru": np.ascontiguousarray(np.asarray(bru, f)[:, r0:r0 + RPC]),
            "wau": np.ascontiguousarray(np.asarray(Wau, f)[h0:h0 + HPC]),
            "bau": np.ascontiguousarray(np.asarray(bau, f)[h0:h0 + HPC]),
            "wvu": np.ascontiguousarray(np.asarray(Wvu, f)[h0:h0 + HPC]),
            "bvu": np.ascontiguousarray(np.asarray(bvu, f)[h0:h0 + HPC]),
            "wvc": np.ascontiguousarray(np.asarray(Wvc, f).reshape(H)),
            "bvc": np.asarray(bvc, f).reshape(1),
        }
        for gi, (base, L) in enumerate(W1GROUPS):
            m[f"w1g{gi}"] = np.ascontiguousarray(
                w1r[base:base + L].transpose(1, 0, 2)).reshape(128, L * 512)
        in_maps.append(m)
    return in_maps


def assemble(results):
    q = np.empty((B, 2 * R + NH * M), np.float32)
    for c in range(NC):
        r0 = c * RPC
        a0 = c * HPC * M
        q[:, r0:r0 + RPC] = results[c]["out_cache"]
        q[:, R + r0:R + r0 + RPC] = results[c]["out_rec"]
        q[:, 2 * R + a0:2 * R + a0 + HPC * M] = results[c]["out_assoc"]
    return q


def run(in_maps, **kw):
    nc = _get_program()
    return bass_utils.run_bass_kernel_spmd(nc, in_maps, core_ids=list(range(NC)), **kw)


def kernel(**inputs):
    in_maps = make_in_maps(**{k: np.asarray(v) for k, v in inputs.items()})
    res = run(in_maps)
    return assemble(res.results)



# revision 4
# speedup vs baseline: 1.9441x; 1.9441x over previous
"""Trainium2 Bass kernel for nn_MultiHeadDuelingDQN (8-core SPMD), v2.

Model (B=256, STATE=26240, H=512, R=4000, N=64 heads, M=10):
    h  = relu(relu(x@W1+b1)@W2+b2)
    q_cache = h@Wvc+bvc + (h@Wac+bac) - mean_R(h@Wac+bac)
    q_assoc = per-head dueling over M (local means)
    q_rec   = S - mean_R(S),  S = h @ W_sum + sum_n bru[n],
              W_sum = sum_n Wru[n]  (exact rewrite; see v1 notes)

v2 redesign vs the f32 baseline (344-438us):
  - All large streams are bf16 (Wru, W1, x, W2, Wac, aug): DMA floor drops
    from ~80MB to ~41MB per core; matmuls run at bf16 PE rate.
  - Host pre-transposes x and lays out W1 so fc1 computes h1T = W1.T x.T
    directly -- zero on-chip transposes (v1 had 60+ PE transposes + copies).
  - Trunk exchange is ReduceScatter(f32, +) then relu+bias+cast and
    AllGather(bf16): the v1 AllToAll measured 96us+39us skew; AG measured
    16.7us. fc2 is then computed replicated (2us of PE).
  - Wru stream: 16 supertiles of [128, 16*500] bf16 (2MB each) on the sync
    HWDGE ring ONLY; DVE does group-of-16 strided X-reduces (bf16 in, f32
    out) + partial adds, final add emits bf16 W_sum for the S matmul.
  - Collective bounce READBACKS are on gpsimd (SWDGE) so HWDGE lanes never
    chain behind ncfw latency (v1's 139us Sync stall).
  - The R-mean exchange is split: adv_c row-sums AllGather fires right after
    the cache head (hidden under the stream; q_cache finalized + written
    early), S row-sums AllGather is the only tail collective.

kernel(**inputs) takes full unsharded inputs, returns full [256, 8640].
"""
import os
os.environ.setdefault("NEURON_RT_DBG_RDH_CC", "0")

import numpy as np
import ml_dtypes

import concourse.bass as bass
import concourse.mybir as mybir
import concourse.tile as tile
from concourse import bacc
from concourse import bass_utils
from concourse.bass import ts

NC = 8
B, H, STATE, R, NH, M = 256, 512, 26240, 4000, 64, 10
KPC_RAW = STATE // NC          # 3280
KCH = 26                       # k-chunks of 128 per core (padded)
KPC = KCH * 128                # 3328
RPC = R // NC                  # 500
HPC = NH // NC                 # 8 heads per core
HS = H // NC                   # 64 h1 rows per core after ReduceScatter
AUG = HPC * (M + 1) + 1        # 89 = [8x(10 adv + 1 val)] + value_c
GRP = 16                       # heads per wru supertile
NGRP = NH // GRP               # 4 supertiles per k-chunk
F32 = mybir.dt.float32
BF16 = mybir.dt.bfloat16
RELU = mybir.ActivationFunctionType.Relu
COPY = mybir.ActivationFunctionType.Copy
IDENT = mybir.ActivationFunctionType.Identity
ADD = mybir.AluOpType.add
RG = [list(range(NC))]


def build_program(wru_bufs=4):
    nc = bacc.Bacc("TRN2", target_bir_lowering=False, debug=False, num_devices=NC)

    # ---- per-core inputs (host-packed layouts, see make_in_maps) ----
    xt = nc.dram_tensor("xt", [128, KCH * B], BF16, kind="ExternalInput").ap()
    w1 = nc.dram_tensor("w1", [128, KCH * H], BF16, kind="ExternalInput").ap()
    w2 = nc.dram_tensor("w2", [128, 4 * H], BF16, kind="ExternalInput").ap()
    b1s = nc.dram_tensor("b1s", [HS, 1], F32, kind="ExternalInput").ap()
    b2c = nc.dram_tensor("b2c", [128, 4], F32, kind="ExternalInput").ap()
    wac = nc.dram_tensor("wac", [128, 4 * RPC], BF16, kind="ExternalInput").ap()
    bac = nc.dram_tensor("bac", [1, RPC], BF16, kind="ExternalInput").ap()
    # [kc, grp, p, r*GRP+gi] = Wru[grp*GRP+gi, kc*128+p, r0+r]
    wru = nc.dram_tensor("wru", [4, NGRP, 128, GRP * RPC], BF16,
                         kind="ExternalInput").ap()
    bru = nc.dram_tensor("bru", [NH, RPC], BF16, kind="ExternalInput").ap()
    aug_w = nc.dram_tensor("aug_w", [128, 4 * AUG], BF16, kind="ExternalInput").ap()
    aug_b = nc.dram_tensor("aug_b", [1, AUG], BF16, kind="ExternalInput").ap()

    out_cache = nc.dram_tensor("out_cache", [B, RPC], F32, kind="ExternalOutput").ap()
    out_rec = nc.dram_tensor("out_rec", [B, RPC], F32, kind="ExternalOutput").ap()
    out_assoc = nc.dram_tensor("out_assoc", [B, HPC * M], F32, kind="ExternalOutput").ap()

    with tile.TileContext(nc) as tc:
        with (
            tc.tile_pool(name="cst", bufs=1) as cst,
            tc.tile_pool(name="sb", bufs=1) as sb,
            tc.tile_pool(name="wrup", bufs=wru_bufs) as wrup,
            tc.tile_pool(name="partp", bufs=2) as partp,
            tc.tile_pool(name="psfc", bufs=4, space="PSUM") as psfc,
            tc.tile_pool(name="psh", bufs=4, space="PSUM") as psh,
            tc.tile_pool(name="dram", bufs=1, space="DRAM") as dram,
        ):
            ones1 = cst.tile([1, 128], BF16, tag="ones1")
            nc.vector.memset(ones1, 1.0)
            ones64 = cst.tile([64, 128], BF16, tag="ones64")
            nc.vector.memset(ones64, 1.0)

            # ---- weight/input loads: scalar (ACT) HWDGE ring, all fast ----
            xt_sb = sb.tile([128, KCH * B], BF16, tag="xt_sb")
            nc.scalar.dma_start(xt_sb, xt)
            w1_sb = sb.tile([128, KCH * H], BF16, tag="w1_sb")
            nc.scalar.dma_start(w1_sb, w1)
            w2_sb = sb.tile([128, 4 * H], BF16, tag="w2_sb")
            nc.scalar.dma_start(w2_sb, w2)
            wac_sb = sb.tile([128, 4 * RPC], BF16, tag="wac_sb")
            nc.scalar.dma_start(wac_sb, wac)
            aug_sb = cst.tile([128, 4 * AUG], BF16, tag="aug_sb")
            nc.scalar.dma_start(aug_sb, aug_w)
            augb_sb = cst.tile([1, AUG], BF16, tag="augb_sb")
            nc.scalar.dma_start(augb_sb, aug_b)
            bac_sb = cst.tile([1, RPC], BF16, tag="bac_sb")
            nc.scalar.dma_start(bac_sb, bac)
            bru_sb = sb.tile([64, RPC], BF16, tag="bru_sb")
            nc.scalar.dma_start(bru_sb, bru)
            b1s_sb = cst.tile([HS, 1], F32, tag="b1s_sb")
            nc.scalar.dma_start(b1s_sb, b1s)
            b2c_sb = cst.tile([128, 4], F32, tag="b2c_sb")
            nc.scalar.dma_start(b2c_sb, b2c)

            # ---- Wru stream (sync HWDGE ring only) + DVE reduction ----
            accf = [sb.tile([128, RPC], F32, tag=f"accf{k}", name=f"accf{k}")
                    for k in range(4)]
            accb = [sb.tile([128, RPC], BF16, tag=f"accb{k}", name=f"accb{k}")
                    for k in range(4)]
            for kc in range(4):
                for g in range(NGRP):
                    wt = wrup.tile([128, GRP * RPC], BF16, tag="wru",
                                   name=f"wru_t{kc}_{g}")
                    nc.sync.dma_start(wt, wru[kc, g])
                    view = bass.AP(wt.tensor, wt.offset,
                                   [wt.ap[0], [GRP, RPC], [1, GRP]])
                    if g == 0:
                        nc.vector.tensor_reduce(accf[kc], view,
                                                axis=mybir.AxisListType.X, op=ADD)
                    else:
                        part = partp.tile([128, RPC], F32, tag="part",
                                          name=f"part{kc}_{g}")
                        nc.vector.tensor_reduce(part, view,
                                                axis=mybir.AxisListType.X, op=ADD)
                        if g < NGRP - 1:
                            nc.vector.tensor_add(accf[kc], accf[kc], part)
                        else:
                            with nc.allow_low_precision(
                                    reason="bf16 W_sum feeds bf16 matmul"):
                                nc.vector.tensor_tensor(
                                    out=accb[kc], in0=accf[kc], in1=part, op=ADD)

            # ---- fc1: h1T partial = W1_slice.T @ x_slice.T  (4 psum banks) ----
            ps1 = [psfc.tile([128, B], F32, tag="fc", name=f"ps1_{jc}")
                   for jc in range(4)]
            for kc in range(KCH):
                for jc in range(4):
                    nc.tensor.matmul(
                        ps1[jc],
                        w1_sb[:, kc * H + jc * 128: kc * H + (jc + 1) * 128],
                        xt_sb[:, kc * B:(kc + 1) * B],
                        start=(kc == 0), stop=(kc == KCH - 1))

            # bounce partials to DRAM (one 512KB write on scalar ring)
            rs_in = dram.tile([H, B], F32, tag="rs_in")
            h1p = sb.tile([128, 4 * B], F32, tag="h1p")
            for jc in range(4):
                nc.scalar.copy(h1p[:, jc * B:(jc + 1) * B], ps1[jc])
            nc.scalar.dma_start(
                rs_in.rearrange("(jc p) b -> p jc b", p=128),
                h1p.rearrange("p (jc b) -> p jc b", b=B))

            # ReduceScatter(+): each core gets its 64 summed h1T rows
            rs_out = dram.tile([HS, B], F32, tag="rs_out")
            nc.gpsimd.collective_compute(
                "ReduceScatter", ADD, replica_groups=RG,
                ins=[rs_in.opt()], outs=[rs_out.opt()])
            rsloc = sb.tile([HS, B], F32, tag="rsloc")
            nc.gpsimd.dma_start(rsloc, rs_out)
            h1loc = sb.tile([HS, B], BF16, tag="h1loc")
            nc.scalar.activation(h1loc, rsloc, RELU, bias=b1s_sb, scale=1.0)

            # AllGather (bf16) -> full h1T, read back as 4 [128,256] chunks
            ag_in = dram.tile([HS, B], BF16, tag="ag_in")
            nc.scalar.dma_start(ag_in, h1loc)
            ag_out = dram.tile([H, B], BF16, tag="ag_out")
            nc.gpsimd.collective_compute(
                "AllGather", mybir.AluOpType.bypass, replica_groups=RG,
                ins=[ag_in.opt()], outs=[ag_out.opt()])
            h1T = sb.tile([128, 4 * B], BF16, tag="h1T")
            nc.gpsimd.dma_start(
                h1T.rearrange("p (kc b) -> p kc b", b=B),
                ag_out.rearrange("(kc p) b -> p kc b", p=128))

            # ---- fc2 (replicated): h2T = relu(W2.T @ h1T + b2) -> hT bf16 ----
            ps2 = [psfc.tile([128, B], F32, tag="fc", name=f"ps2_{jc}")
                   for jc in range(4)]
            for kc in range(4):
                for jc in range(4):
                    nc.tensor.matmul(
                        ps2[jc],
                        w2_sb[:, kc * H + jc * 128: kc * H + (jc + 1) * 128],
                        h1T[:, kc * B:(kc + 1) * B],
                        start=(kc == 0), stop=(kc == 3))
            hT = sb.tile([128, 4 * B], BF16, tag="hT")
            for jc in range(4):
                nc.scalar.activation(hT[:, jc * B:(jc + 1) * B], ps2[jc],
                                     RELU, bias=b2c_sb[:, jc:jc + 1], scale=1.0)

            # ---- assoc heads: augmented [adv | val | value_c] ----
            junkA = sb.tile([128, M], F32, tag="junkA")
            value_sb = []
            for bt in range(2):
                psA = psh.tile([128, AUG], F32, tag="head", name=f"psA{bt}")
                nc.tensor.matmul(psA, ones1, augb_sb, start=True, stop=False)
                for kc in range(4):
                    nc.tensor.matmul(
                        psA, hT[:, kc * B + bt * 128: kc * B + bt * 128 + 128],
                        aug_sb[:, kc * AUG:(kc + 1) * AUG],
                        start=False, stop=(kc == 3))
                psA_sb = sb.tile([128, AUG], F32, tag=f"psAsb{bt}", name=f"psAsb{bt}")
                nc.scalar.copy(psA_sb, psA)
                advs = psA_sb[:, 0:HPC * (M + 1)].rearrange("p (n u) -> p n u", u=M + 1)
                negm = sb.tile([128, HPC], F32, tag=f"negmA{bt}", name=f"negmA{bt}")
                for n in range(HPC):
                    nc.scalar.activation(junkA, advs[:, n, 0:M], COPY,
                                         scale=-1.0 / M,
                                         accum_out=negm[:, n:n + 1])
                tmp = sb.tile([128, HPC], F32, tag=f"tmpA{bt}", name=f"tmpA{bt}")
                nc.gpsimd.tensor_add(tmp, advs[:, :, M], negm)
                q = sb.tile([128, HPC * M], F32, tag=f"qA{bt}", name=f"qA{bt}")
                nc.gpsimd.tensor_tensor(
                    out=q.rearrange("p (n m) -> p n m", m=M),
                    in0=advs[:, :, 0:M],
                    in1=tmp.broadcast_to([128, HPC, M]),
                    op=ADD)
                nc.scalar.dma_start(out_assoc[ts(bt, 128), :], q)
                value_sb.append(psA_sb[:, AUG - 1:AUG])

            # ---- cache head (R-slice) + early row-sum AllGather ----
            ar1 = sb.tile([128, 2], F32, tag="ar1")
            adv_c_sb = []
            for bt in range(2):
                psC = psh.tile([128, RPC], F32, tag="head", name=f"psC{bt}")
                nc.tensor.matmul(psC, ones1, bac_sb, start=True, stop=False)
                for kc in range(4):
                    nc.tensor.matmul(
                        psC, hT[:, kc * B + bt * 128: kc * B + bt * 128 + 128],
                        wac_sb[:, kc * RPC:(kc + 1) * RPC],
                        start=False, stop=(kc == 3))
                t = sb.tile([128, RPC], F32, tag=f"advc{bt}", name=f"advc{bt}")
                nc.scalar.activation(t, psC, COPY, accum_out=ar1[:, bt:bt + 1])
                adv_c_sb.append(t)

            ar1_din = dram.tile([128, 2], F32, tag="ar1_din")
            ar1_dout = dram.tile([NC * 128, 2], F32, tag="ar1_dout")
            nc.scalar.dma_start(ar1_din, ar1)
            nc.gpsimd.collective_compute(
                "AllGather", mybir.AluOpType.bypass, replica_groups=RG,
                ins=[ar1_din.opt()], outs=[ar1_dout.opt()])
            rall1 = sb.tile([128, NC * 2], F32, tag="rall1")
            nc.gpsimd.dma_start(rall1, ar1_dout.rearrange("(g p) c -> p g c", p=128))
            rv1 = bass.AP(rall1.tensor, rall1.offset,
                          [rall1.ap[0], [1, 2], [2, NC]])
            sum1 = sb.tile([128, 2], F32, tag="sum1")
            nc.vector.tensor_reduce(sum1, rv1, axis=mybir.AxisListType.X, op=ADD)
            negm1 = sb.tile([128, 2], F32, tag="negm1")
            nc.scalar.activation(negm1, sum1, COPY, scale=-1.0 / R)
            # q_cache finalized + written EARLY (hidden under the wru stream)
            for bt in range(2):
                vm = sb.tile([128, 1], F32, tag=f"vm{bt}", name=f"vm{bt}")
                nc.gpsimd.tensor_add(vm, value_sb[bt], negm1[:, bt:bt + 1])
                qc = sb.tile([128, RPC], F32, tag=f"qc{bt}", name=f"qc{bt}")
                nc.scalar.activation(qc, adv_c_sb[bt], IDENT, bias=vm, scale=1.0)
                nc.scalar.dma_start(out_cache[ts(bt, 128), :], qc)

            # ---- S head: needs the full wru reduction ----
            ar2 = sb.tile([128, 2], F32, tag="ar2")
            s_sb = []
            for bt in range(2):
                psS = psh.tile([128, RPC], F32, tag="head", name=f"psS{bt}")
                nc.tensor.matmul(psS, ones64, bru_sb, start=True, stop=False)
                for kc in range(4):
                    nc.tensor.matmul(
                        psS, hT[:, kc * B + bt * 128: kc * B + bt * 128 + 128],
                        accb[kc], start=False, stop=(kc == 3))
                st = sb.tile([128, RPC], F32, tag=f"ssb{bt}", name=f"ssb{bt}")
                nc.scalar.activation(st, psS, COPY, accum_out=ar2[:, bt:bt + 1])
                s_sb.append(st)

            # ---- tail: S row-sum AllGather, q_rec finalize ----
            ar2_din = dram.tile([128, 2], F32, tag="ar2_din")
            ar2_dout = dram.tile([NC * 128, 2], F32, tag="ar2_dout")
            nc.scalar.dma_start(ar2_din, ar2)
            nc.gpsimd.collective_compute(
                "AllGather", mybir.AluOpType.bypass, replica_groups=RG,
                ins=[ar2_din.opt()], outs=[ar2_dout.opt()])
            rall2 = sb.tile([128, NC * 2], F32, tag="rall2")
            nc.gpsimd.dma_start(rall2, ar2_dout.rearrange("(g p) c -> p g c", p=128))
            rv2 = bass.AP(rall2.tensor, rall2.offset,
                          [rall2.ap[0], [1, 2], [2, NC]])
            sum2 = sb.tile([128, 2], F32, tag="sum2")
            nc.vector.tensor_reduce(sum2, rv2, axis=mybir.AxisListType.X, op=ADD)
            negm2 = sb.tile([128, 2], F32, tag="negm2")
            nc.scalar.activation(negm2, sum2, COPY, scale=-1.0 / R)
            for bt in range(2):
                qr = sb.tile([128, RPC], F32, tag=f"qr{bt}", name=f"qr{bt}")
                nc.scalar.activation(qr, s_sb[bt], IDENT,
                                     bias=negm2[:, bt:bt + 1], scale=1.0)
                nc.scalar.dma_start(out_rec[ts(bt, 128), :], qr)

    nc.compile()
    return nc


_CACHED = None


def _get_program():
    global _CACHED
    if _CACHED is None:
        _CACHED = build_program()
    return _CACHED


def make_in_maps(x, W1, b1, W2, b2, Wvc, bvc, Wac, bac, Wvu, bvu, Wau, bau, Wru, bru):
    f = np.float32
    bf = ml_dtypes.bfloat16
    x = np.asarray(x, f)
    W1 = np.asarray(W1, f)
    W2 = np.asarray(W2, f)
    Wac = np.asarray(Wac, f)
    Wru = np.asarray(Wru, f)
    Wau = np.asarray(Wau, f)
    Wvu = np.asarray(Wvu, f)
    Wvc = np.asarray(Wvc, f).reshape(H)
    b1 = np.asarray(b1, f)
    b2 = np.asarray(b2, f)
    bac_v = np.asarray(bac, f)
    bau = np.asarray(bau, f)
    bvu = np.asarray(bvu, f)
    bvc = np.asarray(bvc, f).reshape(1)
    bru_m = np.asarray(bru, f)

    w2p = np.ascontiguousarray(
        W2.reshape(4, 128, H).transpose(1, 0, 2)).reshape(128, 4 * H).astype(bf)
    b2cp = np.ascontiguousarray(b2.reshape(4, 128).T)

    in_maps = []
    for c in range(NC):
        k0 = c * KPC_RAW
        xs = np.zeros((KPC, B), f)
        xs[:KPC_RAW] = x[:, k0:k0 + KPC_RAW].T
        xtp = np.ascontiguousarray(
            xs.reshape(KCH, 128, B).transpose(1, 0, 2)).reshape(128, KCH * B).astype(bf)
        w1s = np.zeros((KPC, H), f)
        w1s[:KPC_RAW] = W1[k0:k0 + KPC_RAW]
        w1p = np.ascontiguousarray(
            w1s.reshape(KCH, 128, H).transpose(1, 0, 2)).reshape(128, KCH * H).astype(bf)

        r0 = c * RPC
        h0 = c * HPC
        # wru supertiles: [kc, grp, p, r*GRP+gi] = Wru[grp*GRP+gi, kc*128+p, r0+r]
        ws = Wru[:, :, r0:r0 + RPC]                          # [64, 512, 500]
        a = ws.reshape(NGRP, GRP, 4, 128, RPC)               # [grp, gi, kc, p, r]
        wrup_ = np.ascontiguousarray(a.transpose(2, 0, 3, 4, 1)).reshape(
            4, NGRP, 128, GRP * RPC).astype(bf)

        wacp = np.ascontiguousarray(
            Wac[:, r0:r0 + RPC].reshape(4, 128, RPC).transpose(1, 0, 2)).reshape(
            128, 4 * RPC).astype(bf)

        aug = np.zeros((4, 128, AUG), f)
        ag = aug[:, :, :HPC * (M + 1)].reshape(4, 128, HPC, M + 1)
        ag[:, :, :, :M] = Wau[h0:h0 + HPC].reshape(HPC, 4, 128, M).transpose(1, 2, 0, 3)
        ag[:, :, :, M] = Wvu[h0:h0 + HPC].reshape(HPC, 4, 128).transpose(1, 2, 0)
        aug[:, :, AUG - 1] = Wvc.reshape(4, 128)
        augp = np.ascontiguousarray(aug.transpose(1, 0, 2)).reshape(128, 4 * AUG).astype(bf)
        augb = np.concatenate([
            np.concatenate([bau[h0:h0 + HPC], bvu[h0:h0 + HPC, None]], axis=1).reshape(-1),
            bvc]).reshape(1, AUG).astype(bf)

        m = {
            "xt": xtp, "w1": w1p, "w2": w2p,
            "b1s": np.ascontiguousarray(b1[c * HS:(c + 1) * HS].reshape(HS, 1)),
            "b2c": b2cp,
            "wac": wacp,
            "bac": np.ascontiguousarray(bac_v[r0:r0 + RPC].reshape(1, RPC)).astype(bf),
            "wru": wrup_,
            "bru": np.ascontiguousarray(bru_m[:, r0:r0 + RPC]).astype(bf),
            "aug_w": augp, "aug_b": augb,
        }
        in_maps.append(m)
    return in_maps


def assemble(results):
    q = np.empty((B, 2 * R + NH * M), np.float32)
    for c in range(NC):
        r0 = c * RPC
        a0 = c * HPC * M
        q[:, r0:r0 + RPC] = results[c]["out_cache"]
        q[:, R + r0:R + r0 + RPC] = results[c]["out_rec"]
        q[:, 2 * R + a0:2 * R + a0 + HPC * M] = results[c]["out_assoc"]
    return q


def run(in_maps, **kw):
    nc = _get_program()
    return bass_utils.run_bass_kernel_spmd(nc, in_maps, core_ids=list(range(NC)), **kw)


def kernel(**inputs):
    in_maps = make_in_maps(**{k: np.asarray(v) for k, v in inputs.items()})
    res = run(in_maps)
    return assemble(res.results)
